# revision 35
# baseline (speedup 1.0000x reference)
"""Trainium2 Bass kernel for nn_AttnBlock (linear-attention block).

Full-input contract: kernel(**inputs) takes the complete arrays and returns the
complete output. Internally shards batch B=16 across 8 NeuronCores (2 each).

Math (per batch b, x_b [C=256, N=4096]):
  n1 = LN_C(x);  qkv = Wqkv @ n1;  q,k,v heads of 32
  q = softmax_d(q)/sqrt(32); k = softmax_N(k); v = v/N
  ctx_h = k_h @ v_h^T; out_h = ctx_h^T @ q_h
  y = Wout @ out + bout; out = LN_C(y) + x

v4 design notes:
  - ACT is the critical engine (~45us/batch serial): everything here aims at
    fewer/wider ACT ops and zero dead ACT phases.
  - Table discipline: per batch exactly two table transitions. The Ln block
    (m2(b) + stats(b+1)) is absorbed INTO the y-drain window (Identity is in
    every table set), so no standalone Ln phase and no PE gap at the batch
    boundary; the Exp block covers rsig2(b)/rsig(b+1)/ekt/q.
  - kv in groups of 4 blocks (psKV [128,4,256], 2 banks x 2 bufs) -> 8 ekt
    exps + 8 vts casts per batch instead of 16; stats/m2 drains are 1024-wide
    Lns in the same PSUM ring (same byte size), halving their count.
  - o-phase folded away: M = (ctx_m)^T @ Wout^T once per batch, then
    y = M^T @ (expq/S) directly (1/S commutes through Wout because ctx_m is
    head-block masked).
  - Constants packed into 2 DMAs so the PE warmup isn't gated on a dozen
    serialized const transfers.
  - Batch 0's second C-half loads as f32 on the sync HWDGE queue in parallel
    with the gpsimd casting loads of the first half (lead-in is load-paced);
    DVE casts it to bf16.
  - Output stored in bf16 on the sync queue; host upcasts to f32 (the values
    were already bf16-rounded before the store in every version).
"""

import math
import numpy as np

HEADS = 4
DH = 32
C = 256
N = 4096
B = 16
NCORES = 8
BPC = B // NCORES  # batches per core
EPS = 1e-5
INNER = HEADS * DH  # 128
NB = N // 128       # 32 n-blocks per batch
NCH = 8             # 512-wide chunks
CW = N // NCH       # 512
HW = N // 2         # 2048 half width


def _build_bass():
    import concourse.bass as bass
    import concourse.bacc as bacc
    import concourse.tile as tile
    import concourse.mybir as mybir
    from contextlib import ExitStack

    f32 = mybir.dt.float32
    bf16 = mybir.dt.bfloat16
    AF = mybir.ActivationFunctionType
    MUL = mybir.AluOpType.mult

    nc = bacc.Bacc("TRN2", target_bir_lowering=False, debug=False,
                   num_devices=NCORES)

    # cb = [wq0|wq1|wkv0|wkv1|woct|ones|hind|idm]  (128 x 1408, bf16)
    # cf = [boc0|boc1|bmask]                       (128 x 130, f32)
    xin = nc.dram_tensor("xin", [BPC, C, N], f32, kind="ExternalInput")
    cb = nc.dram_tensor("cb", [128, 1408], bf16, kind="ExternalInput")
    cf = nc.dram_tensor("cf", [128, 130], f32, kind="ExternalInput")
    out = nc.dram_tensor("out", [BPC, C, N], bf16, kind="ExternalOutput")

    with tile.TileContext(nc) as tc, ExitStack() as ctx:
        consts = ctx.enter_context(tc.tile_pool(name="consts", bufs=1))
        xpool = ctx.enter_context(tc.tile_pool(name="xpool", bufs=4))
        sqpool = ctx.enter_context(tc.tile_pool(name="sqpool", bufs=2))
        f32pool = ctx.enter_context(tc.tile_pool(name="f32pool", bufs=1))
        rspool = ctx.enter_context(tc.tile_pool(name="rspool", bufs=1))
        xspool = ctx.enter_context(tc.tile_pool(name="xspool", bufs=2))
        eqpool = ctx.enter_context(tc.tile_pool(name="eqpool", bufs=1))
        kvpool = ctx.enter_context(tc.tile_pool(name="kvpool", bufs=1))
        ycpool = ctx.enter_context(tc.tile_pool(name="ycpool", bufs=2))
        obpool = ctx.enter_context(tc.tile_pool(name="obpool", bufs=2))
        tmppool = ctx.enter_context(tc.tile_pool(name="tmppool", bufs=1))
        statp = ctx.enter_context(tc.tile_pool(name="statp", bufs=2))
        tinyp = ctx.enter_context(tc.tile_pool(name="tinyp", bufs=4))
        psA = ctx.enter_context(tc.tile_pool(name="psA", bufs=3, space="PSUM"))
        psKV = ctx.enter_context(tc.tile_pool(name="psKV", bufs=2, space="PSUM"))
        psC = ctx.enter_context(tc.tile_pool(name="psC", bufs=1, space="PSUM"))

        # constants into SBUF: two DMAs total
        cb_t = consts.tile([128, 1408], bf16, tag="cb")
        nc.sync.dma_start(cb_t[:], cb[:, :])
        cf_t = consts.tile([128, 130], f32, tag="cf")
        nc.sync.dma_start(cf_t[:], cf[:, :])
        wq_t = [cb_t[:, 0:128], cb_t[:, 128:256]]
        wkv_t = [cb_t[:, 256:512], cb_t[:, 512:768]]
        woct_t = cb_t[:, 768:1024]
        ones_t = cb_t[:, 1024:1152]
        hind_t = cb_t[:, 1152:1280]
        idm_t = cb_t[:, 1280:1408]
        boc_t = [cf_t[:, 0:1], cf_t[:, 1:2]]
        bmask_t = cf_t[:, 2:130]
        eps_t = consts.tile([128, 1], f32, tag="eps")
        nc.vector.memset(eps_t[:], EPS)

        # PE warm-up: wait the const DMA once + ramp the PE p-state
        warm_ps = psA.tile([128, 128], f32, tag="pa")
        for t in (wq_t[0], ones_t, hind_t, idm_t):
            nc.tensor.matmul(warm_ps[:, 0:2], t, cb_t[:, 0:2],
                             start=True, stop=True)

        # ---- per-batch state ----
        st = [dict() for _ in range(BPC)]

        # --- stage emitters -------------------------------------------------
        def em_load(b):
            # xa (C 0:128): gpsimd casting DMAs. xb (C 128:256): for batch 0,
            # raw f32 on the parallel sync queue (cast by DVE below); for
            # batch 1, gpsimd casting DMAs (no lead-in pressure).
            s = st[b]
            s["xa"] = xpool.tile([128, N], bf16, tag="x", name=f"xa{b}")
            s["xb"] = xpool.tile([128, N], bf16, tag="x", name=f"xb{b}")
            for lo, hi in ((0, 1024), (1024, 2048), (2048, N)):
                nc.gpsimd.dma_start(s["xa"][:, lo:hi], xin[b, 0:128, lo:hi])
                nc.gpsimd.dma_start(s["xb"][:, lo:hi], xin[b, 128:256, lo:hi])

        def em_xsq(b, i):
            # i in 0..3, 1024-wide
            s = st[b]
            if i == 0:
                s["xsq_a"] = sqpool.tile([128, N], bf16, tag="sq",
                                         name=f"xsqa{b}")
                s["xsq_b"] = sqpool.tile([128, N], bf16, tag="sq",
                                         name=f"xsqb{b}")
            sl = bass.ts(i, 1024)
            nc.vector.tensor_mul(s["xsq_a"][:, sl], s["xa"][:, sl],
                                 s["xa"][:, sl])
            nc.vector.tensor_mul(s["xsq_b"][:, sl], s["xb"][:, sl],
                                 s["xb"][:, sl])

        def em_stats(b, i):
            # i in 0..3: one 1024-wide Ln over a pair of 512 matmul chunks
            s = st[b]
            if i == 0:
                s["lnv"] = f32pool.tile([128, N], bf16, tag="var",
                                        name=f"lnv{b}")
            ps = psKV.tile([128, 1024], f32, tag="kv")
            for j in range(2):
                sl = bass.ts(2 * i + j, CW)
                nc.tensor.matmul(ps[:, 512 * j:512 * (j + 1)], ones_t,
                                 s["xsq_a"][:, sl], start=True, stop=False)
                nc.tensor.matmul(ps[:, 512 * j:512 * (j + 1)], ones_t,
                                 s["xsq_b"][:, sl], start=False, stop=True)
            sl2 = bass.ts(i, 1024)
            nc.scalar.activation(s["lnv"][:, sl2], ps[:], AF.Ln,
                                 bias=eps_t[:])

        def em_rsig_xs(b, h, split=True):
            # rsig = exp(-0.5*lnv); xs = x*rsig. split=False emits ONE
            # full-width exp (h ignored for the exp) so the scheduler cannot
            # hoist a half past pending Lns and thrash the ACT tables.
            s = st[b]
            if h == 0:
                s["rsig"] = rspool.tile([128, N], bf16, tag="rsig",
                                        name=f"rsig{b}")
                s["xs_a"] = xspool.tile([128, N], bf16, tag="xs",
                                        name=f"xsa{b}")
                s["xs_b"] = xspool.tile([128, N], bf16, tag="xs",
                                        name=f"xsb{b}")
            sl = bass.ts(h, HW)
            if split:
                nc.scalar.activation(s["rsig"][:, sl], s["lnv"][:, sl],
                                     AF.Exp, scale=-0.5)
            elif h == 0:
                nc.scalar.activation(s["rsig"][:], s["lnv"][:],
                                     AF.Exp, scale=-0.5)
            nc.vector.tensor_mul(s["xs_a"][:, sl], s["xa"][:, sl],
                                 s["rsig"][:, sl])
            nc.vector.tensor_mul(s["xs_b"][:, sl], s["xb"][:, sl],
                                 s["rsig"][:, sl])

        def em_kv_mm(b, g):
            # g in 0..7: 4 n-blocks per group
            s = st[b]
            if g == 0:
                s["ekt"] = kvpool.tile([128, NB, 128], bf16, tag="ekt",
                                       name=f"ekt{b}")
                s["vts"] = kvpool.tile([128, NB, 129], bf16, tag="vts",
                                       name=f"vts{b}")
                nc.vector.memset(s["vts"][:, :, 128:129], 1.0)
            kv_ps = psKV.tile([128, 4, 256], f32, tag="kv")
            for i in range(4):
                blk = 4 * g + i
                bsl = bass.ts(blk, 128)
                nc.tensor.matmul(kv_ps[:, i, :], s["xs_a"][:, bsl],
                                 wkv_t[0], start=True, stop=False)
                nc.tensor.matmul(kv_ps[:, i, :], s["xs_b"][:, bsl],
                                 wkv_t[1], start=False, stop=True)
            s[f"kvps{g}"] = kv_ps

        def em_kv_drain(b, g):
            s = st[b]
            kv_ps = s.pop(f"kvps{g}")
            nc.scalar.activation(s["ekt"][:, 4 * g:4 * g + 4, :],
                                 kv_ps[:, :, 0:128], AF.Exp)
            nc.vector.tensor_copy(s["vts"][:, 4 * g:4 * g + 4, 0:128],
                                  kv_ps[:, :, 128:256])

        def em_ctx(b, g):
            s = st[b]
            if g == 0:
                s["ctx_ps"] = psC.tile([128, 129], f32, tag="ctx",
                                       name=f"ctx{b}")
            for i in range(4):
                blk = 4 * g + i
                nc.tensor.matmul(s["ctx_ps"][:], s["ekt"][:, blk, :],
                                 s["vts"][:, blk, :],
                                 start=(blk == 0), stop=(blk == NB - 1))

        def em_qp(b, p):
            # p in 0..3: two 512-chunks share each stationary load
            s = st[b]
            if p == 0:
                s["expq"] = eqpool.tile([128, N], bf16, tag="eq",
                                        name=f"expq{b}")
            sl0, sl1 = bass.ts(2 * p, CW), bass.ts(2 * p + 1, CW)
            q0 = psA.tile([128, CW], f32, tag="pa")
            q1 = psA.tile([128, CW], f32, tag="pa")
            nc.tensor.matmul(q0[:], wq_t[0], s["xs_a"][:, sl0],
                             start=True, stop=False)
            nc.tensor.matmul(q1[:], wq_t[0], s["xs_a"][:, sl1],
                             start=True, stop=False)
            nc.tensor.matmul(q0[:], wq_t[1], s["xs_b"][:, sl0],
                             start=False, stop=True)
            nc.tensor.matmul(q1[:], wq_t[1], s["xs_b"][:, sl1],
                             start=False, stop=True)
            nc.scalar.activation(s["expq"][:, sl0], q0[:], AF.Exp)
            nc.scalar.activation(s["expq"][:, sl1], q1[:], AF.Exp)

        def em_ctxfin(b):
            # ctx_m = (ctx/kden)*bmask; M = ctx_m^T @ Wout^T  [128d, 256o]
            s = st[b]
            ctx_ps = s["ctx_ps"]
            # 1/kden commutes past the M matmul (it is per-partition d on
            # both ctx and M), so the recip runs OFF the chain and the final
            # drain applies it on DVE -- one hop shorter, one ACT copy less
            rk = tinyp.tile([128, 1], f32, tag="rk")
            nc.vector.reciprocal(rk[:], ctx_ps[:, 128:129])
            ctx_m = tinyp.tile([128, 128], bf16, tag="cxm")
            nc.vector.tensor_mul(ctx_m[:], ctx_ps[:, 0:128], bmask_t)
            tr_ps = psC.tile([128, 128], bf16, tag="ctx", name=f"tr{b}")
            nc.tensor.transpose(tr_ps[:], ctx_m[:], idm_t)
            ctx_mT = tinyp.tile([128, 128], bf16, tag="cxt")
            nc.scalar.activation(ctx_mT[:], tr_ps[:], AF.Copy)
            m_ps = psC.tile([128, 256], f32, tag="ctx", name=f"mps{b}")
            nc.tensor.matmul(m_ps[:], ctx_mT[:], woct_t,
                             start=True, stop=True)
            s["M"] = tinyp.tile([128, 256], bf16, tag="msb", name=f"msb{b}")
            nc.vector.tensor_scalar_mul(s["M"][:], m_ps[:], rk[:])

        def em_S(b, ch):
            # S = head-sums of expq; eqn = expq / S
            s = st[b]
            if ch == 0:
                s["eqn"] = eqpool.tile([128, N], bf16, tag="eqn",
                                       name=f"eqn{b}")
            sl = bass.ts(ch, CW)
            S_ps = psA.tile([128, CW], f32, tag="pa")
            nc.tensor.matmul(S_ps[:], hind_t, s["expq"][:, sl],
                             start=True, stop=True)
            rS = statp.tile([128, CW], f32, tag="st2")
            nc.vector.reciprocal_approx_fast(rS[:], S_ps[:])
            nc.vector.tensor_mul(s["eqn"][:, sl], s["expq"][:, sl], rS[:])

        def em_y(b, j, ch, dve=False):
            s = st[b]
            if j == 0 and ch == 0:
                s["yc_a"] = ycpool.tile([128, N], bf16, tag="yc",
                                        name=f"yca{b}")
                s["yc_b"] = ycpool.tile([128, N], bf16, tag="yc",
                                        name=f"ycb{b}")
            sl = bass.ts(ch, CW)
            y_ps = psA.tile([128, CW], f32, tag="pa")
            nc.tensor.matmul(y_ps[:], s["M"][:, 128 * j:128 * (j + 1)],
                             s["eqn"][:, sl], start=True, stop=True)
            dst = s["yc_a"] if j == 0 else s["yc_b"]
            if dve:
                # ACT paces the y window; offload some drains to DVE
                nc.vector.tensor_scalar_add(dst[:, sl], y_ps[:], boc_t[j])
            else:
                nc.scalar.activation(dst[:, sl], y_ps[:], AF.Identity,
                                     bias=boc_t[j])

        def em_ysq(b, i):
            # i in 0..3, 1024-wide
            s = st[b]
            if i == 0:
                s["ysq_a"] = sqpool.tile([128, N], bf16, tag="ysq",
                                         name=f"ysqa{b}")
                s["ysq_b"] = sqpool.tile([128, N], bf16, tag="ysq",
                                         name=f"ysqb{b}")
            sl = bass.ts(i, 1024)
            nc.vector.tensor_mul(s["ysq_a"][:, sl], s["yc_a"][:, sl],
                                 s["yc_a"][:, sl])
            nc.vector.tensor_mul(s["ysq_b"][:, sl], s["yc_b"][:, sl],
                                 s["yc_b"][:, sl])

        def em_m2(b, i):
            # i in 0..3: one 1024-wide Ln over a pair of 512 matmul chunks
            s = st[b]
            if i == 0:
                s["lnv2"] = f32pool.tile([128, N], f32, tag="lnv2",
                                         name=f"lnv2{b}")
            ps = psKV.tile([128, 1024], f32, tag="kv")
            for j in range(2):
                sl = bass.ts(2 * i + j, CW)
                nc.tensor.matmul(ps[:, 512 * j:512 * (j + 1)], ones_t,
                                 s["ysq_a"][:, sl], start=True, stop=False)
                nc.tensor.matmul(ps[:, 512 * j:512 * (j + 1)], ones_t,
                                 s["ysq_b"][:, sl], start=False, stop=True)
            sl2 = bass.ts(i, 1024)
            nc.scalar.activation(s["lnv2"][:, sl2], ps[:], AF.Ln,
                                 bias=eps_t[:])

        def em_rsig2(b, h, split=True):
            s = st[b]
            if h == 0:
                s["rsig2"] = rspool.tile([128, N], bf16, tag="rsig2",
                                         name=f"rsig2{b}")
            if split:
                sl = bass.ts(h, HW)
                nc.scalar.activation(s["rsig2"][:, sl], s["lnv2"][:, sl],
                                     AF.Exp, scale=-0.5)
            elif h == 0:
                nc.scalar.activation(s["rsig2"][:], s["lnv2"][:],
                                     AF.Exp, scale=-0.5)

        def em_fin(b, h, part, eng=None):
            # part 0: C 0:128 (a), part 1: C 128:256 (b); 2048-wide
            s = st[b]
            eng = eng if eng is not None else nc.vector
            sl = bass.ts(h, HW)
            yc = s["yc_a"] if part == 0 else s["yc_b"]
            xsrc = s["xa"] if part == 0 else s["xb"]
            csl = slice(0, 128) if part == 0 else slice(128, 256)
            tt = tmppool.tile([128, HW], bf16, tag="tmp")
            ob = obpool.tile([128, HW], bf16, tag="ob")
            eng.tensor_mul(tt[:], yc[:, sl], s["rsig2"][:, sl])
            eng.tensor_add(ob[:], tt[:], xsrc[:, sl])
            nc.sync.dma_start(out[b, csl, sl], ob[:])

        # ---- emission schedule --------------------------------------------
        em_load(0)
        em_load(1)

        # lead-in: xsq + stats for batch 0  [Ln table]
        for i in range(4):
            em_xsq(0, i)
            em_stats(0, i)

        def exp_block(b):
            """Exp-table block for batch b: rsig/xs, kv/ctx (+prev finish on
            DVE), q, ctx->M, S. Ends in the y window with the next Ln block
            interleaved (Identity is table-neutral)."""
            prev = b - 1 if b > 0 else None
            nxt = b + 1 if b + 1 < BPC else None

            em_rsig_xs(b, 0)
            if prev is not None:
                em_rsig2(prev, 0)
            em_rsig_xs(b, 1)
            if prev is not None:
                em_rsig2(prev, 1)

            fin_ops = ([(prev, 0, 0), (prev, 0, 1), (prev, 1, 0),
                        (prev, 1, 1)] if prev is not None else [])
            fi = 0
            for g in range(8):
                em_kv_mm(b, g)
                if g >= 1:
                    em_ctx(b, g - 1)
                em_kv_drain(b, g)
                if g % 2 == 1:
                    # weave q + S into the kv window: their ACT exps and DVE
                    # rS/eqn overlap the kv matmul stream instead of forming
                    # a serial post-kv phase
                    p = g // 2
                    em_qp(b, p)
                    em_S(b, 2 * p)
                    em_S(b, 2 * p + 1)
                elif fi < len(fin_ops) and g in (2, 4):
                    em_fin(*fin_ops[fi]); fi += 1
            em_ctx(b, 7)
            # ctxfin first: its PE transpose + M matmul + ACT copies overlap
            # the remaining prev-batch finish ops on DVE
            em_ctxfin(b)
            while fi < len(fin_ops):
                em_fin(*fin_ops[fi]); fi += 1
            if nxt is not None:
                # next batch's stats depend only on its loads: they fill the
                # ctxfin->M bubble and their Lns complete before the y window
                # (no straggler table thrash)
                for i in range(4):
                    em_xsq(nxt, i)
                    em_stats(nxt, i)

            # y window: Identity drains with the Ln block woven in right
            # behind its producers (ysq/xsq at ch=2i+1 feed m2/stats mm+Ln
            # immediately), so ACT stays saturated in FIFO order and the
            # scheduler has no idle slot to hoist an Exp into (no thrash).
            ndve = 3 if nxt is not None else 5
            for ch in range(NCH):
                em_y(b, 0, ch)
                em_y(b, 1, ch, dve=(ch >= NCH - ndve))
                if ch % 2 == 1:
                    i = ch // 2
                    em_ysq(b, i)
                    em_m2(b, i)

        exp_block(0)
        exp_block(1)
        # tail: finish batch 1
        em_rsig2(1, 0)
        em_rsig2(1, 1)
        em_fin(1, 0, 0)
        em_fin(1, 0, 1)
        em_fin(1, 1, 0)
        em_fin(1, 1, 1)

    nc.compile()
    return nc


_CACHED = {}


def _get_nc():
    if "nc" not in _CACHED:
        _CACHED["nc"] = _build_bass()
    return _CACHED["nc"]


def _make_in_maps(x, Wqkv, Wout, bout):
    import ml_dtypes

    bf = ml_dtypes.bfloat16
    x = np.ascontiguousarray(x, dtype=np.float32)
    Wqkv = np.asarray(Wqkv, dtype=np.float32)
    Wout = np.asarray(Wout, dtype=np.float32)
    bout = np.asarray(bout, dtype=np.float32)

    # host-side weight folding
    Wc = Wqkv - Wqkv.mean(axis=1, keepdims=True)          # centers LN1 input
    wct = np.ascontiguousarray(Wc.T)                      # [256, 384]
    wq = np.ascontiguousarray(wct[:, 0:128]).astype(bf)
    wkv = np.ascontiguousarray(wct[:, 128:384]).astype(bf)
    Woc = Wout - Wout.mean(axis=0, keepdims=True)         # centers LN2 input
    woct = np.ascontiguousarray(Woc.T).astype(bf)         # [128, 256]
    boc = (bout - bout.mean()).reshape(C, 1).astype(np.float32)

    onesc = np.full((128, 128), 1.0 / C, dtype=np.float32).astype(bf)
    r = np.arange(128)
    hindm = (r[:, None] // DH == r[None, :] // DH)
    hind = hindm.astype(bf)
    bmask = hindm.astype(np.float32) * np.float32(1.0 / (N * math.sqrt(DH)))
    idm = np.eye(128, dtype=np.float32).astype(bf)

    # packed constants
    cbm = np.concatenate([wq[0:128], wq[128:256], wkv[0:128], wkv[128:256],
                          woct, onesc, hind, idm], axis=1)
    cbm = np.ascontiguousarray(cbm)
    cfm = np.concatenate([boc[0:128], boc[128:256], bmask], axis=1)
    cfm = np.ascontiguousarray(cfm.astype(np.float32))

    xr = x.reshape(B, C, N)
    in_maps = []
    for core in range(NCORES):
        in_maps.append({
            "xin": np.ascontiguousarray(xr[core * BPC:(core + 1) * BPC]),
            "cb": cbm, "cf": cfm,
        })
    return in_maps


def kernel(x, Wqkv, Wout, bout):
    from concourse.bass_utils import run_bass_kernel_spmd

    nc = _get_nc()
    in_maps = _make_in_maps(x, Wqkv, Wout, bout)
    res = run_bass_kernel_spmd(nc, in_maps, core_ids=list(range(NCORES)))
    outs = [np.asarray(res.results[c]["out"]).astype(np.float32)
            for c in range(NCORES)]
    full = np.concatenate(outs, axis=0).reshape(B, C, 64, 64)
    return full


if __name__ == "__main__":
    rng = np.random.default_rng(0)
    x = rng.standard_normal((B, C, 64, 64), dtype=np.float32)
    Wqkv = rng.standard_normal((384, C), dtype=np.float32)
    Wout = rng.standard_normal((C, INNER), dtype=np.float32)
    bout = rng.standard_normal((C,), dtype=np.float32)
    y = kernel(x=x, Wqkv=Wqkv, Wout=Wout, bout=bout)
    print(y.shape, y.dtype)


# revision 36
# speedup vs baseline: 1.0249x; 1.0249x over previous
"""Trainium2 Bass kernel for nn_AttnBlock (linear-attention block).

Full-input contract: kernel(**inputs) takes the complete arrays and returns the
complete output. Internally shards batch B=16 across 8 NeuronCores (2 each).

Math (per batch b, x_b [C=256, N=4096]):
  n1 = LN_C(x);  qkv = Wqkv @ n1;  q,k,v heads of 32
  q = softmax_d(q)/sqrt(32); k = softmax_N(k); v = v/N
  ctx_h = k_h @ v_h^T; out_h = ctx_h^T @ q_h
  y = Wout @ out + bout; out = LN_C(y) + x

v4 design notes:
  - ACT is the critical engine (~45us/batch serial): everything here aims at
    fewer/wider ACT ops and zero dead ACT phases.
  - Table discipline: per batch exactly two table transitions. The Ln block
    (m2(b) + stats(b+1)) is absorbed INTO the y-drain window (Identity is in
    every table set), so no standalone Ln phase and no PE gap at the batch
    boundary; the Exp block covers rsig2(b)/rsig(b+1)/ekt/q.
  - kv in groups of 4 blocks (psKV [128,4,256], 2 banks x 2 bufs) -> 8 ekt
    exps + 8 vts casts per batch instead of 16; stats/m2 drains are 1024-wide
    Lns in the same PSUM ring (same byte size), halving their count.
  - o-phase folded away: M = (ctx_m)^T @ Wout^T once per batch, then
    y = M^T @ (expq/S) directly (1/S commutes through Wout because ctx_m is
    head-block masked).
  - Constants packed into 2 DMAs so the PE warmup isn't gated on a dozen
    serialized const transfers.
  - Batch 0's second C-half loads as f32 on the sync HWDGE queue in parallel
    with the gpsimd casting loads of the first half (lead-in is load-paced);
    DVE casts it to bf16.
  - Output stored in bf16 on the sync queue; host upcasts to f32 (the values
    were already bf16-rounded before the store in every version).
"""

import math
import numpy as np

HEADS = 4
DH = 32
C = 256
N = 4096
B = 16
NCORES = 8
BPC = B // NCORES  # batches per core
EPS = 1e-5
INNER = HEADS * DH  # 128
NB = N // 128       # 32 n-blocks per batch
NCH = 8             # 512-wide chunks
CW = N // NCH       # 512
HW = N // 2         # 2048 half width


def _build_bass():
    import concourse.bass as bass
    import concourse.bacc as bacc
    import concourse.tile as tile
    import concourse.mybir as mybir
    from contextlib import ExitStack

    f32 = mybir.dt.float32
    bf16 = mybir.dt.bfloat16
    AF = mybir.ActivationFunctionType
    MUL = mybir.AluOpType.mult

    nc = bacc.Bacc("TRN2", target_bir_lowering=False, debug=False,
                   num_devices=NCORES)

    # cb = [wq0|wq1|wkv0|wkv1|woct|ones|hind|idm]  (128 x 1408, bf16)
    # cf = [boc0|boc1|bmask]                       (128 x 130, f32)
    xin = nc.dram_tensor("xin", [BPC, C, N], f32, kind="ExternalInput")
    cb = nc.dram_tensor("cb", [128, 1408], bf16, kind="ExternalInput")
    cf = nc.dram_tensor("cf", [128, 130], f32, kind="ExternalInput")
    out = nc.dram_tensor("out", [BPC, C, N], bf16, kind="ExternalOutput")

    with tile.TileContext(nc) as tc, ExitStack() as ctx:
        consts = ctx.enter_context(tc.tile_pool(name="consts", bufs=1))
        xpool = ctx.enter_context(tc.tile_pool(name="xpool", bufs=4))
        sqpool = ctx.enter_context(tc.tile_pool(name="sqpool", bufs=2))
        f32pool = ctx.enter_context(tc.tile_pool(name="f32pool", bufs=1))
        rspool = ctx.enter_context(tc.tile_pool(name="rspool", bufs=1))
        xspool = ctx.enter_context(tc.tile_pool(name="xspool", bufs=2))
        eqpool = ctx.enter_context(tc.tile_pool(name="eqpool", bufs=1))
        kvpool = ctx.enter_context(tc.tile_pool(name="kvpool", bufs=1))
        ycpool = ctx.enter_context(tc.tile_pool(name="ycpool", bufs=2))
        obpool = ctx.enter_context(tc.tile_pool(name="obpool", bufs=2))
        tmppool = ctx.enter_context(tc.tile_pool(name="tmppool", bufs=1))
        statp = ctx.enter_context(tc.tile_pool(name="statp", bufs=2))
        tinyp = ctx.enter_context(tc.tile_pool(name="tinyp", bufs=4))
        psA = ctx.enter_context(tc.tile_pool(name="psA", bufs=3, space="PSUM"))
        psKV = ctx.enter_context(tc.tile_pool(name="psKV", bufs=2, space="PSUM"))
        psC = ctx.enter_context(tc.tile_pool(name="psC", bufs=1, space="PSUM"))

        # constants into SBUF: two DMAs total
        cb_t = consts.tile([128, 1408], bf16, tag="cb")
        nc.sync.dma_start(cb_t[:], cb[:, :])
        cf_t = consts.tile([128, 130], f32, tag="cf")
        nc.sync.dma_start(cf_t[:], cf[:, :])
        wq_t = [cb_t[:, 0:128], cb_t[:, 128:256]]
        wkv_t = [cb_t[:, 256:512], cb_t[:, 512:768]]
        woct_t = cb_t[:, 768:1024]
        ones_t = cb_t[:, 1024:1152]
        hind_t = cb_t[:, 1152:1280]
        idm_t = cb_t[:, 1280:1408]
        boc_t = [cf_t[:, 0:1], cf_t[:, 1:2]]
        bmask_t = cf_t[:, 2:130]
        eps_t = consts.tile([128, 1], f32, tag="eps")
        nc.vector.memset(eps_t[:], EPS)

        # PE warm-up: wait the const DMA once + ramp the PE p-state
        warm_ps = psA.tile([128, 128], f32, tag="pa")
        for t in (wq_t[0], ones_t, hind_t, idm_t):
            nc.tensor.matmul(warm_ps[:, 0:2], t, cb_t[:, 0:2],
                             start=True, stop=True)

        # ---- per-batch state ----
        st = [dict() for _ in range(BPC)]

        # --- stage emitters -------------------------------------------------
        def em_load(b):
            # xa (C 0:128): gpsimd casting DMAs. xb (C 128:256): for batch 0,
            # raw f32 on the parallel sync queue (cast by DVE below); for
            # batch 1, gpsimd casting DMAs (no lead-in pressure).
            s = st[b]
            s["xa"] = xpool.tile([128, N], bf16, tag="x", name=f"xa{b}")
            s["xb"] = xpool.tile([128, N], bf16, tag="x", name=f"xb{b}")
            for lo, hi in ((0, 1024), (1024, 2048), (2048, N)):
                nc.gpsimd.dma_start(s["xa"][:, lo:hi], xin[b, 0:128, lo:hi])
                nc.gpsimd.dma_start(s["xb"][:, lo:hi], xin[b, 128:256, lo:hi])

        def em_xsq(b, i):
            # i in 0..3, 1024-wide
            s = st[b]
            if i == 0:
                s["xsq_a"] = sqpool.tile([128, N], bf16, tag="sq",
                                         name=f"xsqa{b}")
                s["xsq_b"] = sqpool.tile([128, N], bf16, tag="sq",
                                         name=f"xsqb{b}")
            sl = bass.ts(i, 1024)
            nc.vector.tensor_mul(s["xsq_a"][:, sl], s["xa"][:, sl],
                                 s["xa"][:, sl])
            nc.vector.tensor_mul(s["xsq_b"][:, sl], s["xb"][:, sl],
                                 s["xb"][:, sl])

        def em_stats(b, i):
            # i in 0..3: one 1024-wide Ln over a pair of 512 matmul chunks
            s = st[b]
            if i == 0:
                s["lnv"] = f32pool.tile([128, N], bf16, tag="var",
                                        name=f"lnv{b}")
            ps = psKV.tile([128, 1024], f32, tag="kv")
            for j in range(2):
                sl = bass.ts(2 * i + j, CW)
                nc.tensor.matmul(ps[:, 512 * j:512 * (j + 1)], ones_t,
                                 s["xsq_a"][:, sl], start=True, stop=False)
                nc.tensor.matmul(ps[:, 512 * j:512 * (j + 1)], ones_t,
                                 s["xsq_b"][:, sl], start=False, stop=True)
            sl2 = bass.ts(i, 1024)
            nc.scalar.activation(s["lnv"][:, sl2], ps[:], AF.Ln,
                                 bias=eps_t[:])

        def em_rsig_xs(b, h, split=True):
            # rsig = exp(-0.5*lnv); xs = x*rsig. split=False emits ONE
            # full-width exp (h ignored for the exp) so the scheduler cannot
            # hoist a half past pending Lns and thrash the ACT tables.
            s = st[b]
            if h == 0:
                s["rsig"] = rspool.tile([128, N], bf16, tag="rsig",
                                        name=f"rsig{b}")
                s["xs_a"] = xspool.tile([128, N], bf16, tag="xs",
                                        name=f"xsa{b}")
                s["xs_b"] = xspool.tile([128, N], bf16, tag="xs",
                                        name=f"xsb{b}")
            sl = bass.ts(h, HW)
            if split:
                nc.scalar.activation(s["rsig"][:, sl], s["lnv"][:, sl],
                                     AF.Exp, scale=-0.5)
            elif h == 0:
                nc.scalar.activation(s["rsig"][:], s["lnv"][:],
                                     AF.Exp, scale=-0.5)
            nc.vector.tensor_mul(s["xs_a"][:, sl], s["xa"][:, sl],
                                 s["rsig"][:, sl])
            nc.vector.tensor_mul(s["xs_b"][:, sl], s["xb"][:, sl],
                                 s["rsig"][:, sl])

        def em_kv_mm(b, g):
            # g in 0..7: 4 n-blocks per group
            s = st[b]
            if g == 0:
                s["ekt"] = kvpool.tile([128, NB, 128], bf16, tag="ekt",
                                       name=f"ekt{b}")
                s["vts"] = kvpool.tile([128, NB, 129], bf16, tag="vts",
                                       name=f"vts{b}")
                nc.vector.memset(s["vts"][:, :, 128:129], 1.0)
            kv_ps = psKV.tile([128, 4, 256], f32, tag="kv")
            for i in range(4):
                blk = 4 * g + i
                bsl = bass.ts(blk, 128)
                nc.tensor.matmul(kv_ps[:, i, :], s["xs_a"][:, bsl],
                                 wkv_t[0], start=True, stop=False)
                nc.tensor.matmul(kv_ps[:, i, :], s["xs_b"][:, bsl],
                                 wkv_t[1], start=False, stop=True)
            s[f"kvps{g}"] = kv_ps

        def em_kv_drain(b, g):
            s = st[b]
            kv_ps = s.pop(f"kvps{g}")
            nc.scalar.activation(s["ekt"][:, 4 * g:4 * g + 4, :],
                                 kv_ps[:, :, 0:128], AF.Exp)
            nc.vector.tensor_copy(s["vts"][:, 4 * g:4 * g + 4, 0:128],
                                  kv_ps[:, :, 128:256])

        def em_ctx(b, g):
            s = st[b]
            if g == 0:
                s["ctx_ps"] = psC.tile([128, 129], f32, tag="ctx",
                                       name=f"ctx{b}")
            for i in range(4):
                blk = 4 * g + i
                nc.tensor.matmul(s["ctx_ps"][:], s["ekt"][:, blk, :],
                                 s["vts"][:, blk, :],
                                 start=(blk == 0), stop=(blk == NB - 1))

        def em_qp(b, p):
            # p in 0..3: two 512-chunks share each stationary load
            s = st[b]
            if p == 0:
                s["expq"] = eqpool.tile([128, N], bf16, tag="eq",
                                        name=f"expq{b}")
            sl0, sl1 = bass.ts(2 * p, CW), bass.ts(2 * p + 1, CW)
            q0 = psA.tile([128, CW], f32, tag="pa")
            q1 = psA.tile([128, CW], f32, tag="pa")
            nc.tensor.matmul(q0[:], wq_t[0], s["xs_a"][:, sl0],
                             start=True, stop=False)
            nc.tensor.matmul(q1[:], wq_t[0], s["xs_a"][:, sl1],
                             start=True, stop=False)
            nc.tensor.matmul(q0[:], wq_t[1], s["xs_b"][:, sl0],
                             start=False, stop=True)
            nc.tensor.matmul(q1[:], wq_t[1], s["xs_b"][:, sl1],
                             start=False, stop=True)
            nc.scalar.activation(s["expq"][:, sl0], q0[:], AF.Exp)
            nc.scalar.activation(s["expq"][:, sl1], q1[:], AF.Exp)

        def em_ctxfin(b):
            # ctx_m = (ctx/kden)*bmask; M = ctx_m^T @ Wout^T  [128d, 256o]
            s = st[b]
            ctx_ps = s["ctx_ps"]
            # 1/kden commutes past the M matmul (it is per-partition d on
            # both ctx and M), so the recip runs OFF the chain and the final
            # drain applies it on DVE -- one hop shorter, one ACT copy less
            rk = tinyp.tile([128, 1], f32, tag="rk")
            nc.vector.reciprocal(rk[:], ctx_ps[:, 128:129])
            ctx_m = tinyp.tile([128, 128], bf16, tag="cxm")
            nc.vector.tensor_mul(ctx_m[:], ctx_ps[:, 0:128], bmask_t)
            tr_ps = psC.tile([128, 128], bf16, tag="ctx", name=f"tr{b}")
            nc.tensor.transpose(tr_ps[:], ctx_m[:], idm_t)
            ctx_mT = tinyp.tile([128, 128], bf16, tag="cxt")
            nc.scalar.activation(ctx_mT[:], tr_ps[:], AF.Copy)
            m_ps = psC.tile([128, 256], f32, tag="ctx", name=f"mps{b}")
            nc.tensor.matmul(m_ps[:], ctx_mT[:], woct_t,
                             start=True, stop=True)
            s["M"] = tinyp.tile([128, 256], bf16, tag="msb", name=f"msb{b}")
            nc.vector.tensor_scalar_mul(s["M"][:], m_ps[:], rk[:])

        def em_S(b, ch):
            # S = head-sums of expq; eqn = expq / S
            s = st[b]
            if ch == 0:
                s["eqn"] = eqpool.tile([128, N], bf16, tag="eqn",
                                       name=f"eqn{b}")
            sl = bass.ts(ch, CW)
            S_ps = psA.tile([128, CW], f32, tag="pa")
            nc.tensor.matmul(S_ps[:], hind_t, s["expq"][:, sl],
                             start=True, stop=True)
            rS = statp.tile([128, CW], f32, tag="st2")
            nc.vector.reciprocal_approx_fast(rS[:], S_ps[:])
            nc.vector.tensor_mul(s["eqn"][:, sl], s["expq"][:, sl], rS[:])

        def em_y(b, j, ch, dve=False):
            s = st[b]
            if j == 0 and ch == 0:
                s["yc_a"] = ycpool.tile([128, N], bf16, tag="yc",
                                        name=f"yca{b}")
                s["yc_b"] = ycpool.tile([128, N], bf16, tag="yc",
                                        name=f"ycb{b}")
            sl = bass.ts(ch, CW)
            y_ps = psA.tile([128, CW], f32, tag="pa")
            nc.tensor.matmul(y_ps[:], s["M"][:, 128 * j:128 * (j + 1)],
                             s["eqn"][:, sl], start=True, stop=True)
            dst = s["yc_a"] if j == 0 else s["yc_b"]
            if dve:
                # ACT paces the y window; offload some drains to DVE
                nc.vector.tensor_scalar_add(dst[:, sl], y_ps[:], boc_t[j])
            else:
                nc.scalar.activation(dst[:, sl], y_ps[:], AF.Identity,
                                     bias=boc_t[j])

        def em_ysq(b, i):
            # i in 0..3, 1024-wide
            s = st[b]
            if i == 0:
                s["ysq_a"] = sqpool.tile([128, N], bf16, tag="ysq",
                                         name=f"ysqa{b}")
                s["ysq_b"] = sqpool.tile([128, N], bf16, tag="ysq",
                                         name=f"ysqb{b}")
            sl = bass.ts(i, 1024)
            nc.vector.tensor_mul(s["ysq_a"][:, sl], s["yc_a"][:, sl],
                                 s["yc_a"][:, sl])
            nc.vector.tensor_mul(s["ysq_b"][:, sl], s["yc_b"][:, sl],
                                 s["yc_b"][:, sl])

        def em_m2(b, i):
            # i in 0..3: one 1024-wide Ln over a pair of 512 matmul chunks
            s = st[b]
            if i == 0:
                s["lnv2"] = f32pool.tile([128, N], f32, tag="lnv2",
                                         name=f"lnv2{b}")
            ps = psKV.tile([128, 1024], f32, tag="kv")
            for j in range(2):
                sl = bass.ts(2 * i + j, CW)
                nc.tensor.matmul(ps[:, 512 * j:512 * (j + 1)], ones_t,
                                 s["ysq_a"][:, sl], start=True, stop=False)
                nc.tensor.matmul(ps[:, 512 * j:512 * (j + 1)], ones_t,
                                 s["ysq_b"][:, sl], start=False, stop=True)
            sl2 = bass.ts(i, 1024)
            nc.scalar.activation(s["lnv2"][:, sl2], ps[:], AF.Ln,
                                 bias=eps_t[:])

        def em_rsig2(b, h, split=True):
            s = st[b]
            if h == 0:
                s["rsig2"] = rspool.tile([128, N], bf16, tag="rsig2",
                                         name=f"rsig2{b}")
            if split:
                sl = bass.ts(h, HW)
                nc.scalar.activation(s["rsig2"][:, sl], s["lnv2"][:, sl],
                                     AF.Exp, scale=-0.5)
            elif h == 0:
                nc.scalar.activation(s["rsig2"][:], s["lnv2"][:],
                                     AF.Exp, scale=-0.5)

        def em_fin(b, h, part, eng=None):
            # part 0: C 0:128 (a), part 1: C 128:256 (b); 2048-wide
            s = st[b]
            eng = eng if eng is not None else nc.vector
            sl = bass.ts(h, HW)
            yc = s["yc_a"] if part == 0 else s["yc_b"]
            xsrc = s["xa"] if part == 0 else s["xb"]
            csl = slice(0, 128) if part == 0 else slice(128, 256)
            tt = tmppool.tile([128, HW], bf16, tag="tmp")
            ob = obpool.tile([128, HW], bf16, tag="ob")
            eng.tensor_mul(tt[:], yc[:, sl], s["rsig2"][:, sl])
            eng.tensor_add(ob[:], tt[:], xsrc[:, sl])
            nc.sync.dma_start(out[b, csl, sl], ob[:])

        # ---- emission schedule --------------------------------------------
        em_load(0)
        em_load(1)

        # lead-in: xsq + stats for batch 0  [Ln table]
        for i in range(4):
            em_xsq(0, i)
            em_stats(0, i)

        def exp_block(b):
            """Exp-table block for batch b: rsig/xs, kv/ctx (+prev finish on
            DVE), q, ctx->M, S. Ends in the y window with the next Ln block
            interleaved (Identity is table-neutral)."""
            prev = b - 1 if b > 0 else None
            nxt = b + 1 if b + 1 < BPC else None

            em_rsig_xs(b, 0)
            if prev is not None:
                em_rsig2(prev, 0)
            em_rsig_xs(b, 1)
            if prev is not None:
                em_rsig2(prev, 1)

            fin_ops = ([(prev, 0, 0), (prev, 0, 1), (prev, 1, 0),
                        (prev, 1, 1)] if prev is not None else [])
            fi = 0
            for g in range(8):
                em_kv_mm(b, g)
                if g >= 1:
                    em_ctx(b, g - 1)
                em_kv_drain(b, g)
                if g % 2 == 1:
                    # weave q + S into the kv window: their ACT exps and DVE
                    # rS/eqn overlap the kv matmul stream instead of forming
                    # a serial post-kv phase
                    p = g // 2
                    em_qp(b, p)
                    em_S(b, 2 * p)
                    em_S(b, 2 * p + 1)
                elif fi < len(fin_ops) and g in (2, 4):
                    em_fin(*fin_ops[fi]); fi += 1
            em_ctx(b, 7)
            # ctxfin first: its PE transpose + M matmul + ACT copies overlap
            # the remaining prev-batch finish ops on DVE
            em_ctxfin(b)
            while fi < len(fin_ops):
                em_fin(*fin_ops[fi]); fi += 1
            if nxt is not None:
                # next batch's stats depend only on its loads: they fill the
                # ctxfin->M bubble and their Lns complete before the y window
                # (no straggler table thrash)
                for i in range(4):
                    em_xsq(nxt, i)
                    em_stats(nxt, i)

            # y window: Identity drains with the Ln block woven in right
            # behind its producers (ysq/xsq at ch=2i+1 feed m2/stats mm+Ln
            # immediately), so ACT stays saturated in FIFO order and the
            # scheduler has no idle slot to hoist an Exp into (no thrash).
            ndve = 4 if nxt is not None else 6
            for ch in range(NCH):
                em_y(b, 0, ch)
                em_y(b, 1, ch, dve=(ch >= NCH - ndve))
                if ch % 2 == 1:
                    i = ch // 2
                    em_ysq(b, i)
                    em_m2(b, i)

        exp_block(0)
        exp_block(1)
        # tail: finish batch 1
        em_rsig2(1, 0)
        em_rsig2(1, 1)
        em_fin(1, 0, 0)
        em_fin(1, 0, 1)
        em_fin(1, 1, 0)
        em_fin(1, 1, 1)

    nc.compile()
    return nc


_CACHED = {}


def _get_nc():
    if "nc" not in _CACHED:
        _CACHED["nc"] = _build_bass()
    return _CACHED["nc"]


def _make_in_maps(x, Wqkv, Wout, bout):
    import ml_dtypes

    bf = ml_dtypes.bfloat16
    x = np.ascontiguousarray(x, dtype=np.float32)
    Wqkv = np.asarray(Wqkv, dtype=np.float32)
    Wout = np.asarray(Wout, dtype=np.float32)
    bout = np.asarray(bout, dtype=np.float32)

    # host-side weight folding
    Wc = Wqkv - Wqkv.mean(axis=1, keepdims=True)          # centers LN1 input
    wct = np.ascontiguousarray(Wc.T)                      # [256, 384]
    wq = np.ascontiguousarray(wct[:, 0:128]).astype(bf)
    wkv = np.ascontiguousarray(wct[:, 128:384]).astype(bf)
    Woc = Wout - Wout.mean(axis=0, keepdims=True)         # centers LN2 input
    woct = np.ascontiguousarray(Woc.T).astype(bf)         # [128, 256]
    boc = (bout - bout.mean()).reshape(C, 1).astype(np.float32)

    onesc = np.full((128, 128), 1.0 / C, dtype=np.float32).astype(bf)
    r = np.arange(128)
    hindm = (r[:, None] // DH == r[None, :] // DH)
    hind = hindm.astype(bf)
    bmask = hindm.astype(np.float32) * np.float32(1.0 / (N * math.sqrt(DH)))
    idm = np.eye(128, dtype=np.float32).astype(bf)

    # packed constants
    cbm = np.concatenate([wq[0:128], wq[128:256], wkv[0:128], wkv[128:256],
                          woct, onesc, hind, idm], axis=1)
    cbm = np.ascontiguousarray(cbm)
    cfm = np.concatenate([boc[0:128], boc[128:256], bmask], axis=1)
    cfm = np.ascontiguousarray(cfm.astype(np.float32))

    xr = x.reshape(B, C, N)
    in_maps = []
    for core in range(NCORES):
        in_maps.append({
            "xin": np.ascontiguousarray(xr[core * BPC:(core + 1) * BPC]),
            "cb": cbm, "cf": cfm,
        })
    return in_maps


def kernel(x, Wqkv, Wout, bout):
    from concourse.bass_utils import run_bass_kernel_spmd

    nc = _get_nc()
    in_maps = _make_in_maps(x, Wqkv, Wout, bout)
    res = run_bass_kernel_spmd(nc, in_maps, core_ids=list(range(NCORES)))
    outs = [np.asarray(res.results[c]["out"]).astype(np.float32)
            for c in range(NCORES)]
    full = np.concatenate(outs, axis=0).reshape(B, C, 64, 64)
    return full


if __name__ == "__main__":
    rng = np.random.default_rng(0)
    x = rng.standard_normal((B, C, 64, 64), dtype=np.float32)
    Wqkv = rng.standard_normal((384, C), dtype=np.float32)
    Wout = rng.standard_normal((C, INNER), dtype=np.float32)
    bout = rng.standard_normal((C,), dtype=np.float32)
    y = kernel(x=x, Wqkv=Wqkv, Wout=Wout, bout=bout)
    print(y.shape, y.dtype)


# revision 37
# speedup vs baseline: 1.0258x; 1.0009x over previous
"""Trainium2 Bass kernel for nn_AttnBlock (linear-attention block).

Full-input contract: kernel(**inputs) takes the complete arrays and returns the
complete output. Internally shards batch B=16 across 8 NeuronCores (2 each).

Math (per batch b, x_b [C=256, N=4096]):
  n1 = LN_C(x);  qkv = Wqkv @ n1;  q,k,v heads of 32
  q = softmax_d(q)/sqrt(32); k = softmax_N(k); v = v/N
  ctx_h = k_h @ v_h^T; out_h = ctx_h^T @ q_h
  y = Wout @ out + bout; out = LN_C(y) + x

Design notes (final, ~136us/core vs 158us baseline):
  - ACT and DVE are the balanced critical engines (~95us each): the design
    minimizes ACT instruction count/width overhead and keeps both engines
    saturated through every phase.
  - ACT-table discipline: two Ln<->Exp transitions per batch. The Ln work
    (m2(b) stats) is woven into the y-drain window directly behind its
    producers (Identity lives in every table set), so the readiness-greedy
    tile scheduler never finds an idle ACT slot to hoist an Exp into.
  - o-phase folded away: M = (ctx_m)^T @ Wout^T once per batch, then
    y = M^T @ (expq/S) directly -- 1/S commutes through Wout because ctx_m
    is head-block masked; 1/kden likewise commutes past the M matmul and is
    applied per-partition by DVE off the critical chain.
  - kv in groups of 4 blocks (psKV [128,4,256]) -> 8 ekt exps + 8 vts casts
    per batch; stats/m2 drains are 1024-wide Lns in the same PSUM ring.
  - q/S woven into the kv matmul window; the ctx->M chain latency is filled
    by the next batch's load-independent xsq/stats; prev-batch finish ops
    overlap the kv window on DVE; y-drains split ACT/DVE (4 resp. 6 of 16).
  - Constants packed into 2 DMAs; x loads are chunked gpsimd casting DMAs
    (the single ~220GB/s casting queue paces the lead-in); output stored in
    bf16 on the sync HWDGE queue (host upcasts -- values were already
    bf16-rounded before the store).
"""

import math
import numpy as np

HEADS = 4
DH = 32
C = 256
N = 4096
B = 16
NCORES = 8
BPC = B // NCORES  # batches per core
EPS = 1e-5
INNER = HEADS * DH  # 128
NB = N // 128       # 32 n-blocks per batch
NCH = 8             # 512-wide chunks
CW = N // NCH       # 512
HW = N // 2         # 2048 half width


def _build_bass():
    import concourse.bass as bass
    import concourse.bacc as bacc
    import concourse.tile as tile
    import concourse.mybir as mybir
    from contextlib import ExitStack

    f32 = mybir.dt.float32
    bf16 = mybir.dt.bfloat16
    AF = mybir.ActivationFunctionType
    MUL = mybir.AluOpType.mult

    nc = bacc.Bacc("TRN2", target_bir_lowering=False, debug=False,
                   num_devices=NCORES)

    # cb = [wq0|wq1|wkv0|wkv1|woct|ones|hind|idm]  (128 x 1408, bf16)
    # cf = [boc0|boc1|bmask]                       (128 x 130, f32)
    xin = nc.dram_tensor("xin", [BPC, C, N], f32, kind="ExternalInput")
    cb = nc.dram_tensor("cb", [128, 1408], bf16, kind="ExternalInput")
    cf = nc.dram_tensor("cf", [128, 130], f32, kind="ExternalInput")
    out = nc.dram_tensor("out", [BPC, C, N], bf16, kind="ExternalOutput")

    with tile.TileContext(nc) as tc, ExitStack() as ctx:
        consts = ctx.enter_context(tc.tile_pool(name="consts", bufs=1))
        xpool = ctx.enter_context(tc.tile_pool(name="xpool", bufs=4))
        sqpool = ctx.enter_context(tc.tile_pool(name="sqpool", bufs=2))
        f32pool = ctx.enter_context(tc.tile_pool(name="f32pool", bufs=1))
        rspool = ctx.enter_context(tc.tile_pool(name="rspool", bufs=1))
        xspool = ctx.enter_context(tc.tile_pool(name="xspool", bufs=2))
        eqpool = ctx.enter_context(tc.tile_pool(name="eqpool", bufs=1))
        kvpool = ctx.enter_context(tc.tile_pool(name="kvpool", bufs=1))
        ycpool = ctx.enter_context(tc.tile_pool(name="ycpool", bufs=2))
        obpool = ctx.enter_context(tc.tile_pool(name="obpool", bufs=2))
        tmppool = ctx.enter_context(tc.tile_pool(name="tmppool", bufs=1))
        statp = ctx.enter_context(tc.tile_pool(name="statp", bufs=2))
        tinyp = ctx.enter_context(tc.tile_pool(name="tinyp", bufs=4))
        psA = ctx.enter_context(tc.tile_pool(name="psA", bufs=3, space="PSUM"))
        psKV = ctx.enter_context(tc.tile_pool(name="psKV", bufs=2, space="PSUM"))
        psC = ctx.enter_context(tc.tile_pool(name="psC", bufs=1, space="PSUM"))

        # constants into SBUF: two DMAs total
        cb_t = consts.tile([128, 1408], bf16, tag="cb")
        nc.sync.dma_start(cb_t[:], cb[:, :])
        cf_t = consts.tile([128, 130], f32, tag="cf")
        nc.sync.dma_start(cf_t[:], cf[:, :])
        wq_t = [cb_t[:, 0:128], cb_t[:, 128:256]]
        wkv_t = [cb_t[:, 256:512], cb_t[:, 512:768]]
        woct_t = cb_t[:, 768:1024]
        ones_t = cb_t[:, 1024:1152]
        hind_t = cb_t[:, 1152:1280]
        idm_t = cb_t[:, 1280:1408]
        boc_t = [cf_t[:, 0:1], cf_t[:, 1:2]]
        bmask_t = cf_t[:, 2:130]
        eps_t = consts.tile([128, 1], f32, tag="eps")
        nc.vector.memset(eps_t[:], EPS)

        # PE warm-up: wait the const DMA once + ramp the PE p-state
        warm_ps = psA.tile([128, 128], f32, tag="pa")
        for t in (wq_t[0], ones_t, hind_t, idm_t):
            nc.tensor.matmul(warm_ps[:, 0:2], t, cb_t[:, 0:2],
                             start=True, stop=True)

        # ---- per-batch state ----
        st = [dict() for _ in range(BPC)]

        # --- stage emitters -------------------------------------------------
        def em_load(b):
            # chunked casting DMAs on the gpsimd SWDGE queue (the only
            # cast-capable path; measured ~220GB/s)
            s = st[b]
            s["xa"] = xpool.tile([128, N], bf16, tag="x", name=f"xa{b}")
            s["xb"] = xpool.tile([128, N], bf16, tag="x", name=f"xb{b}")
            for lo, hi in ((0, 1024), (1024, 2048), (2048, N)):
                nc.gpsimd.dma_start(s["xa"][:, lo:hi], xin[b, 0:128, lo:hi])
                nc.gpsimd.dma_start(s["xb"][:, lo:hi], xin[b, 128:256, lo:hi])

        def em_xsq(b, i):
            # i in 0..3, 1024-wide
            s = st[b]
            if i == 0:
                s["xsq_a"] = sqpool.tile([128, N], bf16, tag="sq",
                                         name=f"xsqa{b}")
                s["xsq_b"] = sqpool.tile([128, N], bf16, tag="sq",
                                         name=f"xsqb{b}")
            sl = bass.ts(i, 1024)
            nc.vector.tensor_mul(s["xsq_a"][:, sl], s["xa"][:, sl],
                                 s["xa"][:, sl])
            nc.vector.tensor_mul(s["xsq_b"][:, sl], s["xb"][:, sl],
                                 s["xb"][:, sl])

        def em_stats(b, i):
            # i in 0..3: one 1024-wide Ln over a pair of 512 matmul chunks
            s = st[b]
            if i == 0:
                s["lnv"] = f32pool.tile([128, N], bf16, tag="var",
                                        name=f"lnv{b}")
            ps = psKV.tile([128, 1024], f32, tag="kv")
            for j in range(2):
                sl = bass.ts(2 * i + j, CW)
                nc.tensor.matmul(ps[:, 512 * j:512 * (j + 1)], ones_t,
                                 s["xsq_a"][:, sl], start=True, stop=False)
                nc.tensor.matmul(ps[:, 512 * j:512 * (j + 1)], ones_t,
                                 s["xsq_b"][:, sl], start=False, stop=True)
            sl2 = bass.ts(i, 1024)
            nc.scalar.activation(s["lnv"][:, sl2], ps[:], AF.Ln,
                                 bias=eps_t[:])

        def em_rsig_xs(b, h, split=True):
            # rsig = exp(-0.5*lnv); xs = x*rsig. split=False emits ONE
            # full-width exp (h ignored for the exp) so the scheduler cannot
            # hoist a half past pending Lns and thrash the ACT tables.
            s = st[b]
            if h == 0:
                s["rsig"] = rspool.tile([128, N], bf16, tag="rsig",
                                        name=f"rsig{b}")
                s["xs_a"] = xspool.tile([128, N], bf16, tag="xs",
                                        name=f"xsa{b}")
                s["xs_b"] = xspool.tile([128, N], bf16, tag="xs",
                                        name=f"xsb{b}")
            sl = bass.ts(h, HW)
            if split:
                nc.scalar.activation(s["rsig"][:, sl], s["lnv"][:, sl],
                                     AF.Exp, scale=-0.5)
            elif h == 0:
                nc.scalar.activation(s["rsig"][:], s["lnv"][:],
                                     AF.Exp, scale=-0.5)
            nc.vector.tensor_mul(s["xs_a"][:, sl], s["xa"][:, sl],
                                 s["rsig"][:, sl])
            nc.vector.tensor_mul(s["xs_b"][:, sl], s["xb"][:, sl],
                                 s["rsig"][:, sl])

        def em_kv_mm(b, g):
            # g in 0..7: 4 n-blocks per group
            s = st[b]
            if g == 0:
                s["ekt"] = kvpool.tile([128, NB, 128], bf16, tag="ekt",
                                       name=f"ekt{b}")
                s["vts"] = kvpool.tile([128, NB, 129], bf16, tag="vts",
                                       name=f"vts{b}")
                nc.vector.memset(s["vts"][:, :, 128:129], 1.0)
            kv_ps = psKV.tile([128, 4, 256], f32, tag="kv")
            for i in range(4):
                blk = 4 * g + i
                bsl = bass.ts(blk, 128)
                nc.tensor.matmul(kv_ps[:, i, :], s["xs_a"][:, bsl],
                                 wkv_t[0], start=True, stop=False)
                nc.tensor.matmul(kv_ps[:, i, :], s["xs_b"][:, bsl],
                                 wkv_t[1], start=False, stop=True)
            s[f"kvps{g}"] = kv_ps

        def em_kv_drain(b, g):
            s = st[b]
            kv_ps = s.pop(f"kvps{g}")
            nc.scalar.activation(s["ekt"][:, 4 * g:4 * g + 4, :],
                                 kv_ps[:, :, 0:128], AF.Exp)
            nc.vector.tensor_copy(s["vts"][:, 4 * g:4 * g + 4, 0:128],
                                  kv_ps[:, :, 128:256])

        def em_ctx(b, g):
            s = st[b]
            if g == 0:
                s["ctx_ps"] = psC.tile([128, 129], f32, tag="ctx",
                                       name=f"ctx{b}")
            for i in range(4):
                blk = 4 * g + i
                nc.tensor.matmul(s["ctx_ps"][:], s["ekt"][:, blk, :],
                                 s["vts"][:, blk, :],
                                 start=(blk == 0), stop=(blk == NB - 1))

        def em_qp(b, p):
            # p in 0..3: two 512-chunks share each stationary load
            s = st[b]
            if p == 0:
                s["expq"] = eqpool.tile([128, N], bf16, tag="eq",
                                        name=f"expq{b}")
            sl0, sl1 = bass.ts(2 * p, CW), bass.ts(2 * p + 1, CW)
            q0 = psA.tile([128, CW], f32, tag="pa")
            q1 = psA.tile([128, CW], f32, tag="pa")
            nc.tensor.matmul(q0[:], wq_t[0], s["xs_a"][:, sl0],
                             start=True, stop=False)
            nc.tensor.matmul(q1[:], wq_t[0], s["xs_a"][:, sl1],
                             start=True, stop=False)
            nc.tensor.matmul(q0[:], wq_t[1], s["xs_b"][:, sl0],
                             start=False, stop=True)
            nc.tensor.matmul(q1[:], wq_t[1], s["xs_b"][:, sl1],
                             start=False, stop=True)
            nc.scalar.activation(s["expq"][:, sl0], q0[:], AF.Exp)
            nc.scalar.activation(s["expq"][:, sl1], q1[:], AF.Exp)

        def em_ctxfin(b):
            # ctx_m = (ctx/kden)*bmask; M = ctx_m^T @ Wout^T  [128d, 256o]
            s = st[b]
            ctx_ps = s["ctx_ps"]
            # 1/kden commutes past the M matmul (it is per-partition d on
            # both ctx and M), so the recip runs OFF the chain and the final
            # drain applies it on DVE -- one hop shorter, one ACT copy less
            rk = tinyp.tile([128, 1], f32, tag="rk")
            nc.vector.reciprocal(rk[:], ctx_ps[:, 128:129])
            ctx_m = tinyp.tile([128, 128], bf16, tag="cxm")
            nc.vector.tensor_mul(ctx_m[:], ctx_ps[:, 0:128], bmask_t)
            tr_ps = psC.tile([128, 128], bf16, tag="ctx", name=f"tr{b}")
            nc.tensor.transpose(tr_ps[:], ctx_m[:], idm_t)
            ctx_mT = tinyp.tile([128, 128], bf16, tag="cxt")
            nc.scalar.activation(ctx_mT[:], tr_ps[:], AF.Copy)
            m_ps = psC.tile([128, 256], f32, tag="ctx", name=f"mps{b}")
            nc.tensor.matmul(m_ps[:], ctx_mT[:], woct_t,
                             start=True, stop=True)
            s["M"] = tinyp.tile([128, 256], bf16, tag="msb", name=f"msb{b}")
            nc.vector.tensor_scalar_mul(s["M"][:], m_ps[:], rk[:])

        def em_S(b, ch):
            # S = head-sums of expq; eqn = expq / S
            s = st[b]
            if ch == 0:
                s["eqn"] = eqpool.tile([128, N], bf16, tag="eqn",
                                       name=f"eqn{b}")
            sl = bass.ts(ch, CW)
            S_ps = psA.tile([128, CW], f32, tag="pa")
            nc.tensor.matmul(S_ps[:], hind_t, s["expq"][:, sl],
                             start=True, stop=True)
            rS = statp.tile([128, CW], f32, tag="st2")
            nc.vector.reciprocal_approx_fast(rS[:], S_ps[:])
            nc.vector.tensor_mul(s["eqn"][:, sl], s["expq"][:, sl], rS[:])

        def em_y(b, j, ch, dve=False):
            s = st[b]
            if j == 0 and ch == 0:
                s["yc_a"] = ycpool.tile([128, N], bf16, tag="yc",
                                        name=f"yca{b}")
                s["yc_b"] = ycpool.tile([128, N], bf16, tag="yc",
                                        name=f"ycb{b}")
            sl = bass.ts(ch, CW)
            y_ps = psA.tile([128, CW], f32, tag="pa")
            nc.tensor.matmul(y_ps[:], s["M"][:, 128 * j:128 * (j + 1)],
                             s["eqn"][:, sl], start=True, stop=True)
            dst = s["yc_a"] if j == 0 else s["yc_b"]
            if dve:
                # ACT paces the y window; offload some drains to DVE
                nc.vector.tensor_scalar_add(dst[:, sl], y_ps[:], boc_t[j])
            else:
                nc.scalar.activation(dst[:, sl], y_ps[:], AF.Identity,
                                     bias=boc_t[j])

        def em_ysq(b, i):
            # i in 0..3, 1024-wide
            s = st[b]
            if i == 0:
                s["ysq_a"] = sqpool.tile([128, N], bf16, tag="ysq",
                                         name=f"ysqa{b}")
                s["ysq_b"] = sqpool.tile([128, N], bf16, tag="ysq",
                                         name=f"ysqb{b}")
            sl = bass.ts(i, 1024)
            nc.vector.tensor_mul(s["ysq_a"][:, sl], s["yc_a"][:, sl],
                                 s["yc_a"][:, sl])
            nc.vector.tensor_mul(s["ysq_b"][:, sl], s["yc_b"][:, sl],
                                 s["yc_b"][:, sl])

        def em_m2(b, i):
            # i in 0..3: one 1024-wide Ln over a pair of 512 matmul chunks
            s = st[b]
            if i == 0:
                s["lnv2"] = f32pool.tile([128, N], f32, tag="lnv2",
                                         name=f"lnv2{b}")
            ps = psKV.tile([128, 1024], f32, tag="kv")
            for j in range(2):
                sl = bass.ts(2 * i + j, CW)
                nc.tensor.matmul(ps[:, 512 * j:512 * (j + 1)], ones_t,
                                 s["ysq_a"][:, sl], start=True, stop=False)
                nc.tensor.matmul(ps[:, 512 * j:512 * (j + 1)], ones_t,
                                 s["ysq_b"][:, sl], start=False, stop=True)
            sl2 = bass.ts(i, 1024)
            nc.scalar.activation(s["lnv2"][:, sl2], ps[:], AF.Ln,
                                 bias=eps_t[:])

        def em_rsig2(b, h, split=True):
            s = st[b]
            if h == 0:
                s["rsig2"] = rspool.tile([128, N], bf16, tag="rsig2",
                                         name=f"rsig2{b}")
            if split:
                sl = bass.ts(h, HW)
                nc.scalar.activation(s["rsig2"][:, sl], s["lnv2"][:, sl],
                                     AF.Exp, scale=-0.5)
            elif h == 0:
                nc.scalar.activation(s["rsig2"][:], s["lnv2"][:],
                                     AF.Exp, scale=-0.5)

        def em_fin(b, h, part, eng=None):
            # part 0: C 0:128 (a), part 1: C 128:256 (b); 2048-wide
            s = st[b]
            eng = eng if eng is not None else nc.vector
            sl = bass.ts(h, HW)
            yc = s["yc_a"] if part == 0 else s["yc_b"]
            xsrc = s["xa"] if part == 0 else s["xb"]
            csl = slice(0, 128) if part == 0 else slice(128, 256)
            tt = tmppool.tile([128, HW], bf16, tag="tmp")
            ob = obpool.tile([128, HW], bf16, tag="ob")
            eng.tensor_mul(tt[:], yc[:, sl], s["rsig2"][:, sl])
            eng.tensor_add(ob[:], tt[:], xsrc[:, sl])
            nc.sync.dma_start(out[b, csl, sl], ob[:])

        # ---- emission schedule --------------------------------------------
        em_load(0)
        em_load(1)

        # lead-in: xsq + stats for batch 0  [Ln table]
        for i in range(4):
            em_xsq(0, i)
            em_stats(0, i)

        def exp_block(b):
            """Exp-table block for batch b: rsig/xs, kv/ctx (+prev finish on
            DVE), q, ctx->M, S. Ends in the y window with the next Ln block
            interleaved (Identity is table-neutral)."""
            prev = b - 1 if b > 0 else None
            nxt = b + 1 if b + 1 < BPC else None

            em_rsig_xs(b, 0)
            if prev is not None:
                em_rsig2(prev, 0)
            em_rsig_xs(b, 1)
            if prev is not None:
                em_rsig2(prev, 1)

            fin_ops = ([(prev, 0, 0), (prev, 0, 1), (prev, 1, 0),
                        (prev, 1, 1)] if prev is not None else [])
            fi = 0
            for g in range(8):
                em_kv_mm(b, g)
                if g >= 1:
                    em_ctx(b, g - 1)
                em_kv_drain(b, g)
                if g % 2 == 1:
                    # weave q + S into the kv window: their ACT exps and DVE
                    # rS/eqn overlap the kv matmul stream instead of forming
                    # a serial post-kv phase
                    p = g // 2
                    em_qp(b, p)
                    em_S(b, 2 * p)
                    em_S(b, 2 * p + 1)
                elif fi < len(fin_ops) and g in (2, 4):
                    em_fin(*fin_ops[fi]); fi += 1
            em_ctx(b, 7)
            # ctxfin first: its PE transpose + M matmul + ACT copies overlap
            # the remaining prev-batch finish ops on DVE
            em_ctxfin(b)
            while fi < len(fin_ops):
                em_fin(*fin_ops[fi]); fi += 1
            if nxt is not None:
                # next batch's stats depend only on its loads: they fill the
                # ctxfin->M bubble and their Lns complete before the y window
                # (no straggler table thrash)
                for i in range(4):
                    em_xsq(nxt, i)
                    em_stats(nxt, i)

            # y window: Identity drains with the Ln block woven in right
            # behind its producers (ysq/xsq at ch=2i+1 feed m2/stats mm+Ln
            # immediately), so ACT stays saturated in FIFO order and the
            # scheduler has no idle slot to hoist an Exp into (no thrash).
            ndve = 4 if nxt is not None else 6
            for ch in range(NCH):
                em_y(b, 0, ch)
                em_y(b, 1, ch, dve=(ch >= NCH - ndve))
                if ch % 2 == 1:
                    i = ch // 2
                    em_ysq(b, i)
                    em_m2(b, i)

        exp_block(0)
        exp_block(1)
        # tail: finish batch 1
        em_rsig2(1, 0)
        em_rsig2(1, 1)
        em_fin(1, 0, 0)
        em_fin(1, 0, 1)
        em_fin(1, 1, 0)
        em_fin(1, 1, 1)

    nc.compile()
    return nc


_CACHED = {}


def _get_nc():
    if "nc" not in _CACHED:
        _CACHED["nc"] = _build_bass()
    return _CACHED["nc"]


def _make_in_maps(x, Wqkv, Wout, bout):
    import ml_dtypes

    bf = ml_dtypes.bfloat16
    x = np.ascontiguousarray(x, dtype=np.float32)
    Wqkv = np.asarray(Wqkv, dtype=np.float32)
    Wout = np.asarray(Wout, dtype=np.float32)
    bout = np.asarray(bout, dtype=np.float32)

    # host-side weight folding
    Wc = Wqkv - Wqkv.mean(axis=1, keepdims=True)          # centers LN1 input
    wct = np.ascontiguousarray(Wc.T)                      # [256, 384]
    wq = np.ascontiguousarray(wct[:, 0:128]).astype(bf)
    wkv = np.ascontiguousarray(wct[:, 128:384]).astype(bf)
    Woc = Wout - Wout.mean(axis=0, keepdims=True)         # centers LN2 input
    woct = np.ascontiguousarray(Woc.T).astype(bf)         # [128, 256]
    boc = (bout - bout.mean()).reshape(C, 1).astype(np.float32)

    onesc = np.full((128, 128), 1.0 / C, dtype=np.float32).astype(bf)
    r = np.arange(128)
    hindm = (r[:, None] // DH == r[None, :] // DH)
    hind = hindm.astype(bf)
    bmask = hindm.astype(np.float32) * np.float32(1.0 / (N * math.sqrt(DH)))
    idm = np.eye(128, dtype=np.float32).astype(bf)

    # packed constants
    cbm = np.concatenate([wq[0:128], wq[128:256], wkv[0:128], wkv[128:256],
                          woct, onesc, hind, idm], axis=1)
    cbm = np.ascontiguousarray(cbm)
    cfm = np.concatenate([boc[0:128], boc[128:256], bmask], axis=1)
    cfm = np.ascontiguousarray(cfm.astype(np.float32))

    xr = x.reshape(B, C, N)
    in_maps = []
    for core in range(NCORES):
        in_maps.append({
            "xin": np.ascontiguousarray(xr[core * BPC:(core + 1) * BPC]),
            "cb": cbm, "cf": cfm,
        })
    return in_maps


def kernel(x, Wqkv, Wout, bout):
    from concourse.bass_utils import run_bass_kernel_spmd

    nc = _get_nc()
    in_maps = _make_in_maps(x, Wqkv, Wout, bout)
    res = run_bass_kernel_spmd(nc, in_maps, core_ids=list(range(NCORES)))
    outs = [np.asarray(res.results[c]["out"]).astype(np.float32)
            for c in range(NCORES)]
    full = np.concatenate(outs, axis=0).reshape(B, C, 64, 64)
    return full


if __name__ == "__main__":
    rng = np.random.default_rng(0)
    x = rng.standard_normal((B, C, 64, 64), dtype=np.float32)
    Wqkv = rng.standard_normal((384, C), dtype=np.float32)
    Wout = rng.standard_normal((C, INNER), dtype=np.float32)
    bout = rng.standard_normal((C,), dtype=np.float32)
    y = kernel(x=x, Wqkv=Wqkv, Wout=Wout, bout=bout)
    print(y.shape, y.dtype)
